# revision 17
# baseline (speedup 1.0000x reference)
"""DistMult edge scoring on TRN2 via dma_gather bank-bucketing (fast path).

Edges bucketed by (src_bank, dst_bank), banks of 32768 rows so dma_gather's
int16 in-bank indices work. Each bucket is padded to a multiple of 128 (pad
idx 0 -> harmless garbage rows, scores dropped on host) and gathered with two
dma_gather calls (u, v) on round-robin SWDGE queues. Per-edge W rows are
host-materialized in the gather-interleaved layout and streamed per bucket.
Output layout is dma_gather's partition-interleave; host undoes it.
"""

import sys

sys.path.insert(0, "/opt/trn_rl_repo")

import numpy as np

N_NODES = 500000
N_HID = 64
N_RELS = 10
N_CORES = 8
P = 128
BANK = 32768
NBANK = (N_NODES + BANK - 1) // BANK  # 16
NB = NBANK * NBANK  # 256 buckets


def _build_program2(L, n_bufs=4):
    """L: [NB] per-bucket padded edge counts (multiples of 128, shared by all
    cores)."""
    from contextlib import ExitStack

    from concourse import bass, bacc, mybir

    f32 = mybir.dt.float32
    i16 = mybir.dt.int16

    L = [int(x) for x in L]
    Etot = sum(L)
    CT = Etot // P  # total columns
    col_base = np.concatenate([[0], np.cumsum(L) // P]).astype(int)
    Cmax = max(max(x // P for x in L), 1)
    IDXT = Etot // 16  # idx elements per partition, total

    nc = bacc.Bacc("TRN2", num_swdge_queues=4)
    h = nc.declare_dram_parameter("h", [N_NODES, N_HID], f32, isOutput=False)
    usrc = nc.declare_dram_parameter("usrc", [P, IDXT], i16, isOutput=False)
    vdst = nc.declare_dram_parameter("vdst", [P, IDXT], i16, isOutput=False)
    wt = nc.declare_dram_parameter("wt", [P, CT * N_HID], f32, isOutput=False)
    out = nc.declare_dram_parameter("out", [P, CT], f32, isOutput=True)

    B = n_bufs
    with ExitStack() as es:
        pre = es.enter_context(nc.semaphore("pre"))
        dma_sems = [es.enter_context(nc.semaphore(f"dma{i}")) for i in range(B)]
        ws_sems = [es.enter_context(nc.semaphore(f"ws{i}")) for i in range(B)]
        dve_sem = es.enter_context(nc.semaphore("dve_sem"))
        act_sem = es.enter_context(nc.semaphore("act_sem"))
        st_sem = es.enter_context(nc.semaphore("st_sem"))
        usrc_sb = es.enter_context(nc.sbuf_tensor("usrc_sb", [P, IDXT], i16))
        vdst_sb = es.enter_context(nc.sbuf_tensor("vdst_sb", [P, IDXT], i16))
        scores = es.enter_context(nc.sbuf_tensor("scores", [P, CT], f32))
        u_sb = [
            es.enter_context(nc.sbuf_tensor(f"u{i}", [P, Cmax * N_HID], f32))
            for i in range(B)
        ]
        v_sb = [
            es.enter_context(nc.sbuf_tensor(f"v{i}", [P, Cmax * N_HID], f32))
            for i in range(B)
        ]
        w_sb = [
            es.enter_context(nc.sbuf_tensor(f"w{i}", [P, Cmax * N_HID], f32))
            for i in range(B)
        ]

        buckets = [b for b in range(NB) if L[b] > 0]

        with nc.Block() as block:

            @block.sync
            def _(sync):
                sync.dma_start(out=usrc_sb[:], in_=usrc[:]).then_inc(pre, 16)
                sync.dma_start(out=vdst_sb[:], in_=vdst[:]).then_inc(pre, 16)
                for i, b in enumerate(buckets):
                    s = i % B
                    C = L[b] // P
                    if i >= B:
                        sync.wait_ge(dve_sem, 3 * (i - B + 1))
                    sync.dma_start(
                        out=w_sb[s][:, : C * N_HID],
                        in_=wt[:, col_base[b] * N_HID : (col_base[b] + C) * N_HID],
                    ).then_inc(ws_sems[s], 16)
                sync.wait_ge(act_sem, 1)
                sync.dma_start(out=out[:], in_=scores[:]).then_inc(st_sem, 16)
                sync.wait_ge(st_sem, 16)

            @block.gpsimd
            def _(pool):
                pool.wait_ge(pre, 32)
                for i, b in enumerate(buckets):
                    s = i % B
                    C = L[b] // P
                    sbank, dbank = b // NBANK, b % NBANK
                    sbase = sbank * BANK
                    dbase = dbank * BANK
                    if i >= B:
                        pool.wait_ge(dve_sem, 3 * (i - B + 1))
                    i16ofs = (col_base[b] * P) // 16
                    pool.dma_gather(
                        out_ap=u_sb[s][:, : C * N_HID].rearrange(
                            "p (c d) -> p c d", d=N_HID
                        ),
                        in_ap=h[sbase : min(sbase + BANK, N_NODES), :],
                        idxs_ap=usrc_sb[:, i16ofs : i16ofs + L[b] // 16],
                        num_idxs=L[b],
                        num_idxs_reg=L[b],
                        elem_size=N_HID,
                        single_packet=False,
                        queue_num=(2 * i) % 4,
                    ).then_inc(dma_sems[s], 16)
                    pool.dma_gather(
                        out_ap=v_sb[s][:, : C * N_HID].rearrange(
                            "p (c d) -> p c d", d=N_HID
                        ),
                        in_ap=h[dbase : min(dbase + BANK, N_NODES), :],
                        idxs_ap=vdst_sb[:, i16ofs : i16ofs + L[b] // 16],
                        num_idxs=L[b],
                        num_idxs_reg=L[b],
                        elem_size=N_HID,
                        single_packet=False,
                        queue_num=(2 * i + 1) % 4,
                    ).then_inc(dma_sems[s], 16)

            @block.vector
            def _(dve):
                nd = 0
                for i, b in enumerate(buckets):
                    s = i % B
                    C = L[b] // P
                    dve.wait_ge(dma_sems[s], 32 * (i // B + 1))
                    dve.wait_ge(ws_sems[s], 16 * (i // B + 1))
                    u3 = u_sb[s][:, : C * N_HID].rearrange("p (c d) -> p c d", d=N_HID)
                    dve.tensor_tensor(
                        out=u_sb[s][:, : C * N_HID],
                        in0=u_sb[s][:, : C * N_HID],
                        in1=v_sb[s][:, : C * N_HID],
                        op=mybir.AluOpType.mult,
                    ).then_inc(dve_sem, 1)
                    nd += 1
                    dve.wait_ge(dve_sem, nd)
                    dve.tensor_tensor(
                        out=u3,
                        in0=u3,
                        in1=w_sb[s][:, : C * N_HID].rearrange(
                            "p (c d) -> p c d", d=N_HID
                        ),
                        op=mybir.AluOpType.mult,
                    ).then_inc(dve_sem, 1)
                    nd += 1
                    dve.wait_ge(dve_sem, nd)
                    dve.tensor_reduce(
                        out=scores[:, col_base[b] : col_base[b] + C],
                        in_=u3,
                        axis=mybir.AxisListType.X,
                        op=mybir.AluOpType.add,
                    ).then_inc(dve_sem, 1)
                    nd += 1

            @block.scalar
            def _(act):
                act.wait_ge(dve_sem, 3 * len(buckets))
                act.activation(
                    out=scores[:],
                    in_=scores[:],
                    func=mybir.ActivationFunctionType.Sigmoid,
                ).then_inc(act_sem, 1)

    nc.compile()
    return nc


from concourse import mybir  # noqa: E402  (after sys.path insert)


def _wrap16(vals):
    """[n] -> [16, n/16] wrapped (idx j at (j%16, j//16)), replicated to 128."""
    n = vals.shape[0]
    w = vals.reshape(n // 16, 16).T
    return np.tile(w, (8, 1))


def kernel(h, W, src_idx, dst_idx, rel_idx):
    from concourse.bass_utils import run_bass_kernel_spmd

    h = np.ascontiguousarray(np.asarray(h, dtype=np.float32))
    W = np.ascontiguousarray(np.asarray(W, dtype=np.float32))
    src = np.asarray(src_idx).astype(np.int64)
    dst = np.asarray(dst_idx).astype(np.int64)
    rel = np.asarray(rel_idx).astype(np.int64)

    E = src.shape[0]
    esh = E // N_CORES

    orders, counts_all = [], []
    for i in range(N_CORES):
        sl = slice(i * esh, (i + 1) * esh)
        key = (src[sl] >> 15) * NBANK + (dst[sl] >> 15)
        order = np.argsort(key, kind="stable")
        counts = np.bincount(key, minlength=NB)
        orders.append(order)
        counts_all.append(counts)

    Lmax = np.maximum.reduce(counts_all)
    L = ((Lmax + P - 1) // P) * P  # per-bucket padded length, shared
    Etot = int(L.sum())
    CT = Etot // P
    col_base = np.concatenate([[0], np.cumsum(L) // P]).astype(int)

    in_maps, metas = [], []
    for i in range(N_CORES):
        sl = slice(i * esh, (i + 1) * esh)
        s_sh, d_sh, r_sh = src[sl], dst[sl], rel[sl]
        order, counts = orders[i], counts_all[i]
        s_loc = np.zeros(Etot, dtype=np.int16)
        d_loc = np.zeros(Etot, dtype=np.int16)
        r_pad = np.zeros(Etot, dtype=np.int64)
        origpos = np.full(Etot, -1, dtype=np.int64)
        off = 0
        for b in range(NB):
            c = int(counts[b])
            base = col_base[b] * P
            if c:
                idxs = order[off : off + c]
                off += c
                s_loc[base : base + c] = (s_sh[idxs] - (b // NBANK) * BANK).astype(
                    np.int16
                )
                d_loc[base : base + c] = (d_sh[idxs] - (b % NBANK) * BANK).astype(
                    np.int16
                )
                r_pad[base : base + c] = r_sh[idxs]
                origpos[base : base + c] = idxs
        # interleaved layouts
        usrc = _wrap16(s_loc)
        vdst = _wrap16(d_loc)
        # rel grid in (p, col) layout: slot j -> (j%128, j//128)
        rel_grid = r_pad.reshape(CT, P).T  # [128, CT]
        wt = np.ascontiguousarray(W[rel_grid].reshape(P, CT * N_HID))
        in_maps.append({"h": h, "usrc": usrc, "vdst": vdst, "wt": wt})
        metas.append(origpos)

    key = tuple(int(x) for x in L)
    if key not in _PROGRAM_CACHE:
        _PROGRAM_CACHE[key] = _build_program2(L)
    nc = _PROGRAM_CACHE[key]

    res = run_bass_kernel_spmd(
        nc, in_maps, core_ids=list(range(N_CORES)), trace=TRACE
    )
    global LAST_RESULT
    LAST_RESULT = res

    out_full = np.empty(E, dtype=np.float32)
    for i in range(N_CORES):
        arr = np.asarray(res.results[i]["out"])  # [128, CT]
        s_lin = arr.T.reshape(-1)  # slot j = (j%128, j//128) -> arr[p, c]
        origpos = metas[i]
        m = origpos >= 0
        out_full[i * esh + origpos[m]] = s_lin[m]
    return out_full


_PROGRAM_CACHE = {}
TRACE = False
LAST_RESULT = None


# revision 18
# speedup vs baseline: 1.0319x; 1.0319x over previous
"""DistMult edge scoring on TRN2 via dma_gather bank-bucketing (fast path).

Edges bucketed by (src_bank, dst_bank), banks of 32768 rows so dma_gather's
int16 in-bank indices work. Each bucket is padded to a multiple of 128 (pad
idx 0 -> harmless garbage rows, scores dropped on host) and gathered with two
dma_gather calls (u, v) on round-robin SWDGE queues. Per-edge W rows are
host-materialized in the gather-interleaved layout and streamed per bucket.
Output layout is dma_gather's partition-interleave; host undoes it.
"""

import sys

sys.path.insert(0, "/opt/trn_rl_repo")

import numpy as np

N_NODES = 500000
N_HID = 64
N_RELS = 10
N_CORES = 8
P = 128
BANK = 32768
NBANK = (N_NODES + BANK - 1) // BANK  # 16
NB = NBANK * NBANK  # 256 buckets


def _build_program2(L, n_bufs=6):
    """L: [NB] per-bucket padded edge counts (multiples of 128, shared by all
    cores)."""
    from contextlib import ExitStack

    from concourse import bass, bacc, mybir

    f32 = mybir.dt.float32
    i16 = mybir.dt.int16

    L = [int(x) for x in L]
    Etot = sum(L)
    CT = Etot // P  # total columns
    col_base = np.concatenate([[0], np.cumsum(L) // P]).astype(int)
    Cmax = max(max(x // P for x in L), 1)
    IDXT = Etot // 16  # idx elements per partition, total

    nc = bacc.Bacc("TRN2", num_swdge_queues=4)
    h = nc.declare_dram_parameter("h", [N_NODES, N_HID], f32, isOutput=False)
    usrc = nc.declare_dram_parameter("usrc", [P, IDXT], i16, isOutput=False)
    vdst = nc.declare_dram_parameter("vdst", [P, IDXT], i16, isOutput=False)
    wt = nc.declare_dram_parameter("wt", [P, CT * N_HID], f32, isOutput=False)
    out = nc.declare_dram_parameter("out", [P, CT], f32, isOutput=True)

    B = n_bufs
    with ExitStack() as es:
        pre = es.enter_context(nc.semaphore("pre"))
        dma_sems = [es.enter_context(nc.semaphore(f"dma{i}")) for i in range(B)]
        ws_sems = [es.enter_context(nc.semaphore(f"ws{i}")) for i in range(B)]
        dve_sem = es.enter_context(nc.semaphore("dve_sem"))
        act_sem = es.enter_context(nc.semaphore("act_sem"))
        st_sem = es.enter_context(nc.semaphore("st_sem"))
        usrc_sb = es.enter_context(nc.sbuf_tensor("usrc_sb", [P, IDXT], i16))
        vdst_sb = es.enter_context(nc.sbuf_tensor("vdst_sb", [P, IDXT], i16))
        scores = es.enter_context(nc.sbuf_tensor("scores", [P, CT], f32))
        u_sb = [
            es.enter_context(nc.sbuf_tensor(f"u{i}", [P, Cmax * N_HID], f32))
            for i in range(B)
        ]
        v_sb = [
            es.enter_context(nc.sbuf_tensor(f"v{i}", [P, Cmax * N_HID], f32))
            for i in range(B)
        ]
        w_sb = [
            es.enter_context(nc.sbuf_tensor(f"w{i}", [P, Cmax * N_HID], f32))
            for i in range(B)
        ]

        buckets = [b for b in range(NB) if L[b] > 0]

        with nc.Block() as block:

            @block.sync
            def _(sync):
                sync.dma_start(out=usrc_sb[:], in_=usrc[:]).then_inc(pre, 16)
                sync.dma_start(out=vdst_sb[:], in_=vdst[:]).then_inc(pre, 16)
                for i, b in enumerate(buckets):
                    s = i % B
                    C = L[b] // P
                    if i >= B:
                        sync.wait_ge(dve_sem, 3 * (i - B + 1))
                    sync.dma_start(
                        out=w_sb[s][:, : C * N_HID],
                        in_=wt[:, col_base[b] * N_HID : (col_base[b] + C) * N_HID],
                    ).then_inc(ws_sems[s], 16)
                sync.wait_ge(act_sem, 1)
                sync.dma_start(out=out[:], in_=scores[:]).then_inc(st_sem, 16)
                sync.wait_ge(st_sem, 16)

            @block.gpsimd
            def _(pool):
                pool.wait_ge(pre, 32)
                for i, b in enumerate(buckets):
                    s = i % B
                    C = L[b] // P
                    sbank, dbank = b // NBANK, b % NBANK
                    sbase = sbank * BANK
                    dbase = dbank * BANK
                    if i >= B:
                        pool.wait_ge(dve_sem, 3 * (i - B + 1))
                    i16ofs = (col_base[b] * P) // 16
                    pool.dma_gather(
                        out_ap=u_sb[s][:, : C * N_HID].rearrange(
                            "p (c d) -> p c d", d=N_HID
                        ),
                        in_ap=h[sbase : min(sbase + BANK, N_NODES), :],
                        idxs_ap=usrc_sb[:, i16ofs : i16ofs + L[b] // 16],
                        num_idxs=L[b],
                        num_idxs_reg=L[b],
                        elem_size=N_HID,
                        single_packet=False,
                        queue_num=(2 * i) % 4,
                    ).then_inc(dma_sems[s], 16)
                    pool.dma_gather(
                        out_ap=v_sb[s][:, : C * N_HID].rearrange(
                            "p (c d) -> p c d", d=N_HID
                        ),
                        in_ap=h[dbase : min(dbase + BANK, N_NODES), :],
                        idxs_ap=vdst_sb[:, i16ofs : i16ofs + L[b] // 16],
                        num_idxs=L[b],
                        num_idxs_reg=L[b],
                        elem_size=N_HID,
                        single_packet=False,
                        queue_num=(2 * i + 1) % 4,
                    ).then_inc(dma_sems[s], 16)

            @block.vector
            def _(dve):
                nd = 0
                for i, b in enumerate(buckets):
                    s = i % B
                    C = L[b] // P
                    dve.wait_ge(dma_sems[s], 32 * (i // B + 1))
                    dve.wait_ge(ws_sems[s], 16 * (i // B + 1))
                    u3 = u_sb[s][:, : C * N_HID].rearrange("p (c d) -> p c d", d=N_HID)
                    dve.tensor_tensor(
                        out=u_sb[s][:, : C * N_HID],
                        in0=u_sb[s][:, : C * N_HID],
                        in1=v_sb[s][:, : C * N_HID],
                        op=mybir.AluOpType.mult,
                    ).then_inc(dve_sem, 1)
                    nd += 1
                    dve.wait_ge(dve_sem, nd)
                    dve.tensor_tensor(
                        out=u3,
                        in0=u3,
                        in1=w_sb[s][:, : C * N_HID].rearrange(
                            "p (c d) -> p c d", d=N_HID
                        ),
                        op=mybir.AluOpType.mult,
                    ).then_inc(dve_sem, 1)
                    nd += 1
                    dve.wait_ge(dve_sem, nd)
                    dve.tensor_reduce(
                        out=scores[:, col_base[b] : col_base[b] + C],
                        in_=u3,
                        axis=mybir.AxisListType.X,
                        op=mybir.AluOpType.add,
                    ).then_inc(dve_sem, 1)
                    nd += 1

            @block.scalar
            def _(act):
                act.wait_ge(dve_sem, 3 * len(buckets))
                act.activation(
                    out=scores[:],
                    in_=scores[:],
                    func=mybir.ActivationFunctionType.Sigmoid,
                ).then_inc(act_sem, 1)

    nc.compile()
    return nc


from concourse import mybir  # noqa: E402  (after sys.path insert)


def _wrap16(vals):
    """[n] -> [16, n/16] wrapped (idx j at (j%16, j//16)), replicated to 128."""
    n = vals.shape[0]
    w = vals.reshape(n // 16, 16).T
    return np.tile(w, (8, 1))


def kernel(h, W, src_idx, dst_idx, rel_idx):
    from concourse.bass_utils import run_bass_kernel_spmd

    h = np.ascontiguousarray(np.asarray(h, dtype=np.float32))
    W = np.ascontiguousarray(np.asarray(W, dtype=np.float32))
    src = np.asarray(src_idx).astype(np.int64)
    dst = np.asarray(dst_idx).astype(np.int64)
    rel = np.asarray(rel_idx).astype(np.int64)

    E = src.shape[0]
    esh = E // N_CORES

    orders, counts_all = [], []
    for i in range(N_CORES):
        sl = slice(i * esh, (i + 1) * esh)
        key = (src[sl] >> 15) * NBANK + (dst[sl] >> 15)
        order = np.argsort(key, kind="stable")
        counts = np.bincount(key, minlength=NB)
        orders.append(order)
        counts_all.append(counts)

    Lmax = np.maximum.reduce(counts_all)
    L = ((Lmax + P - 1) // P) * P  # per-bucket padded length, shared
    Etot = int(L.sum())
    CT = Etot // P
    col_base = np.concatenate([[0], np.cumsum(L) // P]).astype(int)

    in_maps, metas = [], []
    for i in range(N_CORES):
        sl = slice(i * esh, (i + 1) * esh)
        s_sh, d_sh, r_sh = src[sl], dst[sl], rel[sl]
        order, counts = orders[i], counts_all[i]
        s_loc = np.zeros(Etot, dtype=np.int16)
        d_loc = np.zeros(Etot, dtype=np.int16)
        r_pad = np.zeros(Etot, dtype=np.int64)
        origpos = np.full(Etot, -1, dtype=np.int64)
        off = 0
        for b in range(NB):
            c = int(counts[b])
            base = col_base[b] * P
            if c:
                idxs = order[off : off + c]
                off += c
                s_loc[base : base + c] = (s_sh[idxs] - (b // NBANK) * BANK).astype(
                    np.int16
                )
                d_loc[base : base + c] = (d_sh[idxs] - (b % NBANK) * BANK).astype(
                    np.int16
                )
                r_pad[base : base + c] = r_sh[idxs]
                origpos[base : base + c] = idxs
        # interleaved layouts
        usrc = _wrap16(s_loc)
        vdst = _wrap16(d_loc)
        # rel grid in (p, col) layout: slot j -> (j%128, j//128)
        rel_grid = r_pad.reshape(CT, P).T  # [128, CT]
        wt = np.ascontiguousarray(W[rel_grid].reshape(P, CT * N_HID))
        in_maps.append({"h": h, "usrc": usrc, "vdst": vdst, "wt": wt})
        metas.append(origpos)

    key = tuple(int(x) for x in L)
    if key not in _PROGRAM_CACHE:
        _PROGRAM_CACHE[key] = _build_program2(L)
    nc = _PROGRAM_CACHE[key]

    res = run_bass_kernel_spmd(
        nc, in_maps, core_ids=list(range(N_CORES)), trace=TRACE
    )
    global LAST_RESULT
    LAST_RESULT = res

    out_full = np.empty(E, dtype=np.float32)
    for i in range(N_CORES):
        arr = np.asarray(res.results[i]["out"])  # [128, CT]
        s_lin = arr.T.reshape(-1)  # slot j = (j%128, j//128) -> arr[p, c]
        origpos = metas[i]
        m = origpos >= 0
        out_full[i * esh + origpos[m]] = s_lin[m]
    return out_full


_PROGRAM_CACHE = {}
TRACE = False
LAST_RESULT = None


# revision 19
# speedup vs baseline: 1.1164x; 1.0819x over previous
"""DistMult edge scoring on TRN2 via dma_gather bank-bucketing (fast path).

Edges bucketed by (src_bank, dst_bank), banks of 32768 rows so dma_gather's
int16 in-bank indices work. Each bucket is padded to a multiple of 128 (pad
idx 0 -> harmless garbage rows, scores dropped on host) and gathered with two
dma_gather calls (u, v) on round-robin SWDGE queues. Per-edge W rows are
host-materialized in the gather-interleaved layout and streamed per bucket.
Output layout is dma_gather's partition-interleave; host undoes it.
"""

import sys

sys.path.insert(0, "/opt/trn_rl_repo")

import numpy as np

N_NODES = 500000
N_HID = 64
N_RELS = 10
N_CORES = 8
P = 128
BANK = 32768
NBANK = (N_NODES + BANK - 1) // BANK  # 16
NB = NBANK * NBANK  # 256 buckets


def _build_program2(L, L16, n_bufs=6):
    """L: [NB] per-bucket padded edge counts (multiples of 128, shared by all
    cores)."""
    from contextlib import ExitStack

    from concourse import bass, bacc, mybir

    f32 = mybir.dt.float32
    i16 = mybir.dt.int16

    L = [int(x) for x in L]
    L16 = [int(x) for x in L16]
    Etot = sum(L)
    CT = Etot // P  # total columns
    col_base = np.concatenate([[0], np.cumsum(L) // P]).astype(int)
    Cmax = max(max(x // P for x in L), 1)
    IDXT = Etot // 16  # idx elements per partition, total

    nc = bacc.Bacc("TRN2", num_swdge_queues=4)
    h = nc.declare_dram_parameter("h", [N_NODES, N_HID], f32, isOutput=False)
    usrc = nc.declare_dram_parameter("usrc", [P, IDXT], i16, isOutput=False)
    vdst = nc.declare_dram_parameter("vdst", [P, IDXT], i16, isOutput=False)
    wt = nc.declare_dram_parameter("wt", [P, CT * N_HID], f32, isOutput=False)
    out = nc.declare_dram_parameter("out", [P, CT], f32, isOutput=True)

    B = n_bufs
    with ExitStack() as es:
        pre = es.enter_context(nc.semaphore("pre"))
        dma_sems = [es.enter_context(nc.semaphore(f"dma{i}")) for i in range(B)]
        ws_sems = [es.enter_context(nc.semaphore(f"ws{i}")) for i in range(B)]
        dve_sem = es.enter_context(nc.semaphore("dve_sem"))
        act_sem = es.enter_context(nc.semaphore("act_sem"))
        st_sem = es.enter_context(nc.semaphore("st_sem"))
        usrc_sb = es.enter_context(nc.sbuf_tensor("usrc_sb", [P, IDXT], i16))
        vdst_sb = es.enter_context(nc.sbuf_tensor("vdst_sb", [P, IDXT], i16))
        scores = es.enter_context(nc.sbuf_tensor("scores", [P, CT], f32))
        u_sb = [
            es.enter_context(nc.sbuf_tensor(f"u{i}", [P, Cmax * N_HID], f32))
            for i in range(B)
        ]
        v_sb = [
            es.enter_context(nc.sbuf_tensor(f"v{i}", [P, Cmax * N_HID], f32))
            for i in range(B)
        ]
        w_sb = [
            es.enter_context(nc.sbuf_tensor(f"w{i}", [P, Cmax * N_HID], f32))
            for i in range(B)
        ]

        buckets = [b for b in range(NB) if L[b] > 0]

        with nc.Block() as block:

            @block.sync
            def _(sync):
                sync.dma_start(out=usrc_sb[:], in_=usrc[:]).then_inc(pre, 16)
                sync.dma_start(out=vdst_sb[:], in_=vdst[:]).then_inc(pre, 16)
                for i, b in enumerate(buckets):
                    s = i % B
                    C = L[b] // P
                    if i >= B:
                        sync.wait_ge(dve_sem, 3 * (i - B + 1))
                    sync.dma_start(
                        out=w_sb[s][:, : C * N_HID],
                        in_=wt[:, col_base[b] * N_HID : (col_base[b] + C) * N_HID],
                    ).then_inc(ws_sems[s], 16)
                sync.wait_ge(act_sem, 1)
                sync.dma_start(out=out[:], in_=scores[:]).then_inc(st_sem, 16)
                sync.wait_ge(st_sem, 16)

            @block.gpsimd
            def _(pool):
                pool.wait_ge(pre, 32)
                for i, b in enumerate(buckets):
                    s = i % B
                    C = L[b] // P
                    sbank, dbank = b // NBANK, b % NBANK
                    sbase = sbank * BANK
                    dbase = dbank * BANK
                    if i >= B:
                        pool.wait_ge(dve_sem, 3 * (i - B + 1))
                    i16ofs = (col_base[b] * P) // 16
                    pool.dma_gather(
                        out_ap=u_sb[s][:, : C * N_HID].rearrange(
                            "p (c d) -> p c d", d=N_HID
                        ),
                        in_ap=h[sbase : min(sbase + BANK, N_NODES), :],
                        idxs_ap=usrc_sb[:, i16ofs : i16ofs + L16[b] // 16],
                        num_idxs=L16[b],
                        num_idxs_reg=L16[b],
                        elem_size=N_HID,
                        single_packet=False,
                        queue_num=(2 * i) % 4,
                    ).then_inc(dma_sems[s], 16)
                    pool.dma_gather(
                        out_ap=v_sb[s][:, : C * N_HID].rearrange(
                            "p (c d) -> p c d", d=N_HID
                        ),
                        in_ap=h[dbase : min(dbase + BANK, N_NODES), :],
                        idxs_ap=vdst_sb[:, i16ofs : i16ofs + L16[b] // 16],
                        num_idxs=L16[b],
                        num_idxs_reg=L16[b],
                        elem_size=N_HID,
                        single_packet=False,
                        queue_num=(2 * i + 1) % 4,
                    ).then_inc(dma_sems[s], 16)

            @block.vector
            def _(dve):
                nd = 0
                for i, b in enumerate(buckets):
                    s = i % B
                    C = L[b] // P
                    dve.wait_ge(dma_sems[s], 32 * (i // B + 1))
                    dve.wait_ge(ws_sems[s], 16 * (i // B + 1))
                    u3 = u_sb[s][:, : C * N_HID].rearrange("p (c d) -> p c d", d=N_HID)
                    dve.tensor_tensor(
                        out=u_sb[s][:, : C * N_HID],
                        in0=u_sb[s][:, : C * N_HID],
                        in1=v_sb[s][:, : C * N_HID],
                        op=mybir.AluOpType.mult,
                    ).then_inc(dve_sem, 1)
                    nd += 1
                    dve.wait_ge(dve_sem, nd)
                    dve.tensor_tensor(
                        out=u3,
                        in0=u3,
                        in1=w_sb[s][:, : C * N_HID].rearrange(
                            "p (c d) -> p c d", d=N_HID
                        ),
                        op=mybir.AluOpType.mult,
                    ).then_inc(dve_sem, 1)
                    nd += 1
                    dve.wait_ge(dve_sem, nd)
                    dve.tensor_reduce(
                        out=scores[:, col_base[b] : col_base[b] + C],
                        in_=u3,
                        axis=mybir.AxisListType.X,
                        op=mybir.AluOpType.add,
                    ).then_inc(dve_sem, 1)
                    nd += 1

            @block.scalar
            def _(act):
                act.wait_ge(dve_sem, 3 * len(buckets))
                act.activation(
                    out=scores[:],
                    in_=scores[:],
                    func=mybir.ActivationFunctionType.Sigmoid,
                ).then_inc(act_sem, 1)

    nc.compile()
    return nc


from concourse import mybir  # noqa: E402  (after sys.path insert)


def _wrap16(vals):
    """[n] -> [16, n/16] wrapped (idx j at (j%16, j//16)), replicated to 128."""
    n = vals.shape[0]
    w = vals.reshape(n // 16, 16).T
    return np.tile(w, (8, 1))


def kernel(h, W, src_idx, dst_idx, rel_idx):
    from concourse.bass_utils import run_bass_kernel_spmd

    h = np.ascontiguousarray(np.asarray(h, dtype=np.float32))
    W = np.ascontiguousarray(np.asarray(W, dtype=np.float32))
    src = np.asarray(src_idx).astype(np.int64)
    dst = np.asarray(dst_idx).astype(np.int64)
    rel = np.asarray(rel_idx).astype(np.int64)

    E = src.shape[0]
    esh = E // N_CORES

    orders, counts_all = [], []
    for i in range(N_CORES):
        sl = slice(i * esh, (i + 1) * esh)
        key = (src[sl] >> 15) * NBANK + (dst[sl] >> 15)
        order = np.argsort(key, kind="stable")
        counts = np.bincount(key, minlength=NB)
        orders.append(order)
        counts_all.append(counts)

    Lmax = np.maximum.reduce(counts_all)
    L = ((Lmax + P - 1) // P) * P  # per-bucket padded length, shared
    # descriptors actually issued per bucket: only 16-aligned, not 128
    L16 = np.where(Lmax > 0, ((Lmax + 15) // 16) * 16, 0)
    Etot = int(L.sum())
    CT = Etot // P
    col_base = np.concatenate([[0], np.cumsum(L) // P]).astype(int)

    in_maps, metas = [], []
    for i in range(N_CORES):
        sl = slice(i * esh, (i + 1) * esh)
        s_sh, d_sh, r_sh = src[sl], dst[sl], rel[sl]
        order, counts = orders[i], counts_all[i]
        s_loc = np.zeros(Etot, dtype=np.int16)
        d_loc = np.zeros(Etot, dtype=np.int16)
        r_pad = np.zeros(Etot, dtype=np.int64)
        origpos = np.full(Etot, -1, dtype=np.int64)
        off = 0
        for b in range(NB):
            c = int(counts[b])
            base = col_base[b] * P
            if c:
                idxs = order[off : off + c]
                off += c
                s_loc[base : base + c] = (s_sh[idxs] - (b // NBANK) * BANK).astype(
                    np.int16
                )
                d_loc[base : base + c] = (d_sh[idxs] - (b % NBANK) * BANK).astype(
                    np.int16
                )
                r_pad[base : base + c] = r_sh[idxs]
                origpos[base : base + c] = idxs
        # interleaved layouts
        usrc = _wrap16(s_loc)
        vdst = _wrap16(d_loc)
        # rel grid in (p, col) layout: slot j -> (j%128, j//128)
        rel_grid = r_pad.reshape(CT, P).T  # [128, CT]
        wt = np.ascontiguousarray(W[rel_grid].reshape(P, CT * N_HID))
        in_maps.append({"h": h, "usrc": usrc, "vdst": vdst, "wt": wt})
        metas.append(origpos)

    key = tuple(int(x) for x in L) + tuple(int(x) for x in L16)
    if key not in _PROGRAM_CACHE:
        _PROGRAM_CACHE[key] = _build_program2(L, L16)
    nc = _PROGRAM_CACHE[key]

    res = run_bass_kernel_spmd(
        nc, in_maps, core_ids=list(range(N_CORES)), trace=TRACE
    )
    global LAST_RESULT
    LAST_RESULT = res

    out_full = np.empty(E, dtype=np.float32)
    for i in range(N_CORES):
        arr = np.asarray(res.results[i]["out"])  # [128, CT]
        s_lin = arr.T.reshape(-1)  # slot j = (j%128, j//128) -> arr[p, c]
        origpos = metas[i]
        m = origpos >= 0
        out_full[i * esh + origpos[m]] = s_lin[m]
    return out_full


_PROGRAM_CACHE = {}
TRACE = False
LAST_RESULT = None
